# revision 11
# baseline (speedup 1.0000x reference)
"""Causal self-attention (B=4, T=2048, C=1024, 16 heads) on 8 Trainium2 cores.

Sharding: batch x head-group.  Core c handles batch b = c//2 and head group
hg = c%2 (8 heads = 4 head-pairs).  Each core computes q/k/v projections for
its heads, causal attention, and a partial output projection; the host sums
the two partials per batch at gather.

v2 design (vs v1 at ~355-440us):
  * qkv/V projections run in float32r straight from DMA'd f32 inputs -- no
    staging or rounding copies for x and w_qkv (f32r matmuls are 1 cyc/row
    at N=512).  Attention stays fp16 (diag tiles have N=128 where f32r is
    4 cyc/row).
  * V is produced directly in [t, ch] layout by swapping matmul operands
    (lhsT = x^T tile), eliminating the PE transposes and vtmp copies of v1.
  * Per-si softmax: one 2-bank PSUM tile holds both heads' score strips;
    ONE activation instruction (2-region strided AP) does exp for both
    heads, halving ACT instruction overhead.
  * Causal mask: multiplicative fp16 0/1 mask applied post-exp on the SBUF
    probability tile (safe: scaled scores stay well under fp16 max), much
    cheaper than the f32 PSUM additive mask and off the score->exp chain.
  * Softmax denominators ride along as ones-columns in the V tiles
    ([onesA | VA | VB | onesB]); normalization uses GpSimd
    partition_broadcast (no DRAM round trip as in v1), unblocking the
    single-buffered y PSUM banks ~3us earlier per pair.
  * Software pipelining: the PE executes in emission order, so B(j+1)
    qkv groups and proj(j-1) groups are emitted interleaved between
    attention(j) si-groups to fill the PE bubbles left by the exp
    dependency (ACT is the per-si rate limiter during attention).
  * PSUM: qkv/V/proj share a 2-bank rotation, scores 2x2 banks, y 2 banks.

v4: cross-rep software pipelining.  attention(3) has 64 exp-bound si-groups
but the only legal same-rep filler is proj(<=2) (all qkv must precede it),
leaving the PE ~20us idle there.  With multiple reps in the NEFF (the
measured steady state, like consecutive transformer layers), rep r+1's
x/weight DMAs and B(0) q-projection groups are emitted into attention(3,r)'s
bubbles; k/V(0) groups follow during pair 3 once their WAR hazards clear
(earlier pairs still read kT(0)/vAB(0..3) at their si 0..3).  All reps share
one pool scope so persistent tiles chain rep-to-rep by WAR subtile deps.
The ones-column/mask init is also hoisted out of the rep loop.
"""
import numpy as np

import concourse.bass as bass
import concourse.tile as tile
from concourse import mybir, bacc
from concourse.bass_utils import run_bass_kernel_spmd

f32 = mybir.dt.float32
f32r = mybir.dt.float32r
f16 = mybir.dt.float16
Exp = mybir.ActivationFunctionType.Exp

B, T, C = 4, 2048, 1024
N_HEAD = 16
D = C // N_HEAD                 # 64
HPC = N_HEAD // 2               # heads per core = 8
NPAIR = HPC // 2                # head pairs per core = 4
CO_Q = C // 2                   # q channels per core = 512
CT = C // 128                   # contraction tiles for qkv = 8
TJ = T // 512                   # t super-tiles = 4
NS = T // 128                   # s tiles = 16
SCALE = float(D) ** -0.5        # 0.125

_CACHE = {}

# A/B knobs: mask: "mult" (fp16 post-exp multiplicative) | "add" (f32
# pre-exp additive); xrep: cross-rep pipelining of B(0) into attention(3).
OPTS = {"mask": "mult", "xrep": True}


def _reg2(t, col0, width, span=512):
    """2-region AP: columns [col0:col0+width] and [span+col0:span+col0+width]
    of a [128, 2*span] tile."""
    base = t[:, col0:col0 + 1]
    return bass.AP(tensor=base.tensor, offset=base.offset,
                   ap=[t.ap[0], [span, 2], [1, width]])


class _RepState:
    def __init__(self, rep):
        self.rep = rep
        self.xr = {}
        self.y = {}
        self.o = {}


def _build_nc(reps=1):
    import contextlib

    nc = bacc.Bacc("TRN2", target_bir_lowering=False, debug=False)
    xT_d = nc.dram_tensor("xT", [C, T], f32r, kind="ExternalInput").ap()
    wqT_d = nc.dram_tensor("wqT", [C, CO_Q], f32r, kind="ExternalInput").ap()
    wkT_d = nc.dram_tensor("wkT", [C, CO_Q], f32r, kind="ExternalInput").ap()
    wvT_d = nc.dram_tensor("wvT", [C, CO_Q], f32r, kind="ExternalInput").ap()
    wpT_d = nc.dram_tensor("wpT", [CO_Q, C], f32, kind="ExternalInput").ap()
    bmask_d = nc.dram_tensor("bmask", [128, 128], f16, kind="ExternalInput").ap()
    amask_d = nc.dram_tensor("amask", [128, 128], f32, kind="ExternalInput").ap()
    out_d = nc.dram_tensor("out", [T, C], f32, kind="ExternalOutput").ap()

    def dma(out, in_):
        nc.sync.dma_start(out=out, in_=in_)

    with tile.TileContext(nc) as tc, contextlib.ExitStack() as ctx:
        ep = ctx.enter_context
        persist = ep(tc.tile_pool(name="persist", bufs=1))
        xin = ep(tc.tile_pool(name="xin", bufs=2))
        wstg = ep(tc.tile_pool(name="wstg", bufs=2))
        pw = ep(tc.tile_pool(name="pw", bufs=3))
        ypool = ep(tc.tile_pool(name="ypool", bufs=2))
        rp = ep(tc.tile_pool(name="rp", bufs=1))
        ob = ep(tc.tile_pool(name="ob", bufs=2))
        qvp = ep(tc.tile_pool(name="qvp", bufs=2, space="PSUM"))
        sps = ep(tc.tile_pool(name="sps", bufs=2, space="PSUM"))
        ypsp = ep(tc.tile_pool(name="ypsp", bufs=1, space="PSUM"))

        # ---- persistent tensors, shared by all reps (WAR-chained) ----
        qT = persist.tile([128, NPAIR, TJ, 512], f16)
        kT = persist.tile([128, NPAIR, TJ, 512], f16)
        # vAB[:, p, si, :] = [onesA(64) | VA(64) | VB(64) | onesB(64)]
        vAB = persist.tile([128, NPAIR, NS, 256], f16)
        bmask = persist.tile([128, 128], f16)
        amask = persist.tile([128, 128], f32)
        wq_sb = persist.tile([128, CT, CO_Q], f32r)
        wk_sb = persist.tile([128, CT, CO_Q], f32r)
        wv_sb = persist.tile([128, CT, CO_Q], f32r)
        wpT_r = persist.tile([128, NPAIR, C], f16)
        onecol = persist.tile([128, 64], f16)

        # ---- one-time init (masks, ones-columns) ----
        dma(out=bmask[:], in_=bmask_d[:, :])
        if OPTS["mask"] == "add":
            dma(out=amask[:], in_=amask_d[:, :])
        nc.vector.memset(onecol[:], 0.0)
        nc.vector.memset(onecol[:, 0:1], 1.0)
        tmpl = bass.AP(tensor=onecol.tensor, offset=onecol.offset,
                       ap=[onecol.ap[0], [0, NPAIR], [0, NS], onecol.ap[1]])
        nc.vector.tensor_copy(vAB[:, :, :, 0:64], tmpl)
        nc.vector.tensor_copy(vAB[:, :, :, 192:256], tmpl)

        def emit_x_dma(s, j):
            xr = xin.tile([128, CT, 512], f32r, tag="xr", name=f"xr{s.rep}{j}")
            src = bass.AP(tensor=xT_d.tensor, offset=j * 512,
                          ap=[[T, 128], [128 * T, CT], [1, 512]])
            dma(out=xr[:], in_=src)
            s.xr[j] = xr

        def setup_dma_pieces(s):
            def wdmas():
                for wsb, wd in ((wq_sb, wqT_d), (wk_sb, wkT_d), (wv_sb, wvT_d)):
                    src = bass.AP(tensor=wd.tensor, offset=0,
                                  ap=[[CO_Q, 128], [128 * CO_Q, CT], [1, CO_Q]])
                    dma(out=wsb[:], in_=src)
            return [lambda: emit_x_dma(s, 0), wdmas]

        def wp_stage_pieces(s):
            pieces = []
            for p in range(NPAIR):
                def f(p=p):
                    ws = wstg.tile([128, C], f32, tag="ws", name=f"ws{s.rep}{p}")
                    dma(out=ws[:], in_=wpT_d[p * 128:(p + 1) * 128, :])
                    nc.vector.tensor_copy(wpT_r[:, p, :], ws[:])
                pieces.append(f)
            return pieces

        def b_group_pieces(s, j, kind, p_or_sj):
            """Closures for one qkv/V group: 4 matmul chunks + evict."""
            pieces = []
            if kind in ("q", "k"):
                i, dst, p = p_or_sj
                holder = {}

                def mk(ct0):
                    def f():
                        if ct0 == 0:
                            holder["ps"] = qvp.tile([128, 512], f32, tag="qv",
                                                    name=f"qk{s.rep}{j}{p}")
                        ps = holder["ps"]
                        wsb = (wq_sb, wk_sb)[i]
                        for ct in (ct0, ct0 + 1):
                            nc.tensor.matmul(
                                ps[:], wsb[:, ct, p * 128:(p + 1) * 128],
                                s.xr[j][:, ct, :],
                                start=(ct == 0), stop=(ct == CT - 1))
                    return f
                for ct0 in range(0, CT, 2):
                    pieces.append(mk(ct0))

                def ev():
                    nc.vector.tensor_copy(dst[:, p, j, :], holder["ps"][:])
                pieces.append(ev)
            else:  # V group: out [t-block, 512 vch]
                sj = p_or_sj
                holder = {}

                def mkv(ct0):
                    def f():
                        if ct0 == 0:
                            holder["ps"] = qvp.tile([128, 512], f32, tag="qv",
                                                    name=f"v{s.rep}{j}{sj}")
                        ps = holder["ps"]
                        for ct in (ct0, ct0 + 1):
                            nc.tensor.matmul(
                                ps[:], s.xr[j][:, ct, sj * 128:(sj + 1) * 128],
                                wv_sb[:, ct, :],
                                start=(ct == 0), stop=(ct == CT - 1))
                    return f
                for ct0 in range(0, CT, 2):
                    pieces.append(mkv(ct0))

                def evv():
                    si = j * 4 + sj
                    ps = holder["ps"]
                    for p in range(NPAIR):
                        nc.vector.tensor_copy(
                            vAB[:, p, si, 64:192], ps[:, p * 128:(p + 1) * 128])
                pieces.append(evv)
            return pieces

        def phase_b_pieces(s, j):
            pieces = [lambda: emit_x_dma(s, j)]
            for p in range(NPAIR):
                pieces += b_group_pieces(s, j, "q", (0, qT, p))
                pieces += b_group_pieces(s, j, "k", (1, kT, p))
            for sj in range(4):
                pieces += b_group_pieces(s, j, "v", sj)
            return pieces

        def b0_q_pieces(s):
            pieces = []
            for p in range(NPAIR):
                pieces += b_group_pieces(s, 0, "q", (0, qT, p))
            return pieces

        def b0_kv_pieces(s):
            pieces = []
            for p in range(NPAIR):
                pieces += b_group_pieces(s, 0, "k", (1, kT, p))
            for sj in range(4):
                pieces += b_group_pieces(s, 0, "v", sj)
            return pieces

        def proj_pieces(s, j):
            pieces = []
            for tj in range(4):
                for nh in range(2):
                    holder = {}

                    def mkp(p0, tj=tj, nh=nh, holder=holder):
                        def f():
                            if p0 == 0:
                                holder["ps"] = qvp.tile(
                                    [128, 512], f32, tag="qv",
                                    name=f"pr{s.rep}{j}{tj}{nh}")
                                if nh == 0:
                                    s.o[(j, tj)] = ob.tile(
                                        [128, C], f32, tag="o",
                                        name=f"o{s.rep}{j}{tj}")
                            ps = holder["ps"]
                            Y = s.y[j]
                            for p in (p0, p0 + 1):
                                nc.tensor.matmul(
                                    ps[:], Y[:, p, tj * 128:(tj + 1) * 128],
                                    wpT_r[:, p, nh * 512:(nh + 1) * 512],
                                    start=(p == 0), stop=(p == NPAIR - 1))
                        return f
                    pieces.append(mkp(0))
                    pieces.append(mkp(2))

                    def evp(tj=tj, nh=nh, holder=holder):
                        o_sb = s.o[(j, tj)]
                        nc.vector.tensor_copy(
                            o_sb[:, nh * 512:(nh + 1) * 512], holder["ps"][:])
                        if nh == 1:
                            row = j * 512 + tj * 128
                            dma(out=out_d[row:row + 128, :], in_=o_sb[:])
                    pieces.append(evp)
            return pieces

        def attention(s, j, fillers, tails):
            nsj = 4 * (j + 1)
            nslot = NPAIR * (nsj + 1)
            # tail fillers run only in pair 3 from si>=4 (their WAR hazards
            # against this rep's early-si reads clear there)
            ntail_slots = nsj - 4 + 1
            fi = 0
            ti = 0

            def pop(lst, idx, slots_left):
                want = len(lst) - idx
                if want <= 0:
                    return idx
                n = -(-want // max(slots_left, 1)) if slots_left > 0 else want
                for _ in range(n):
                    if idx < len(lst):
                        lst[idx]()
                        idx += 1
                return idx
            slot = nslot
            tslot = ntail_slots
            Y = ypool.tile([128, NPAIR, 512], f16, tag="Y", name=f"Y{s.rep}{j}")
            s.y[j] = Y
            for p in range(NPAIR):
                ypsA = ypsp.tile([128, 512], f32, tag="ypsA")
                ypsB = ypsp.tile([128, 512], f32, tag="ypsB")
                for si in range(nsj):
                    rel = si * 128 - j * 512
                    lo = max(rel, 0)
                    w = 512 - lo
                    stAB = sps.tile([128, 1024], f32, tag="st")
                    ko, ks = si // 4, (si % 4) * 128
                    nc.tensor.matmul(
                        stAB[:, lo:512], kT[0:64, p, ko, ks:ks + 128],
                        qT[0:64, p, j, lo:512], start=True, stop=True)
                    nc.tensor.matmul(
                        stAB[:, 512 + lo:1024], kT[64:128, p, ko, ks:ks + 128],
                        qT[64:128, p, j, lo:512], start=True, stop=True)
                    if rel >= 0 and OPTS["mask"] == "add":
                        mreg = _reg2(stAB, lo, 128)
                        msrc = bass.AP(tensor=amask.tensor, offset=amask.offset,
                                       ap=[amask.ap[0], [0, 2], [1, 128]])
                        nc.vector.tensor_add(mreg, mreg, msrc)
                    pAB = pw.tile([128, 1024], f16, tag="p")
                    nc.scalar.activation(_reg2(pAB, lo, w), _reg2(stAB, lo, w),
                                         Exp, scale=SCALE)
                    if rel >= 0 and OPTS["mask"] == "mult":
                        mreg = _reg2(pAB, lo, 128)
                        msrc = bass.AP(tensor=bmask.tensor, offset=bmask.offset,
                                       ap=[bmask.ap[0], [0, 2], [1, 128]])
                        nc.vector.tensor_mul(mreg, mreg, msrc)
                    st_f = (si == 0)
                    sp_f = (si == nsj - 1)
                    nc.tensor.matmul(ypsA[:, lo:512], vAB[:, p, si, 0:128],
                                     pAB[:, lo:512], start=st_f, stop=sp_f)
                    nc.tensor.matmul(ypsB[:, lo:512], vAB[:, p, si, 128:256],
                                     pAB[:, 512 + lo:1024], start=st_f, stop=sp_f)
                    slot -= 1
                    fi = pop(fillers, fi, slot)
                    if p == NPAIR - 1 and si >= 4:
                        tslot -= 1
                        ti = pop(tails, ti, tslot)
                # normalize: denomA at ypsA row 0, denomB at ypsB row 64
                rtA = rp.tile([1, 512], f32, tag="rtA")
                rtB = rp.tile([1, 512], f32, tag="rtB")
                nc.vector.reciprocal(rtA[0:1, :], ypsA[0:1, :])
                nc.vector.reciprocal(rtB[0:1, :], ypsB[64:65, :])
                rbA = rp.tile([128, 512], f32, tag="rbA")
                rbB = rp.tile([128, 512], f32, tag="rbB")
                nc.gpsimd.partition_broadcast(rbA[:], rtA[0:1, :])
                nc.gpsimd.partition_broadcast(rbB[:], rtB[0:1, :])
                nc.vector.tensor_mul(Y[64:128, p, :], ypsA[64:128, :],
                                     rbA[64:128, :])
                nc.vector.tensor_mul(Y[0:64, p, :], ypsB[0:64, :], rbB[0:64, :])
                slot -= 1
                fi = pop(fillers, fi, slot)
            while fi < len(fillers):
                fillers[fi]()
                fi += 1
            while ti < len(tails):
                tails[ti]()
                ti += 1

        # ---- schedule ----
        states = [_RepState(r) for r in range(reps)]
        for r in range(reps):
            s = states[r]
            if r == 0:
                for piece in setup_dma_pieces(s) + wp_stage_pieces(s):
                    piece()
                for piece in b0_q_pieces(s) + b0_kv_pieces(s):
                    piece()
            else:
                for piece in wp_stage_pieces(s):
                    piece()
            for j in range(TJ):
                fillers, tails = [], []
                if j + 1 < TJ:
                    fillers += phase_b_pieces(s, j + 1)
                if j - 1 >= 0:
                    fillers += proj_pieces(s, j - 1)
                if j == TJ - 1 and r + 1 < reps and OPTS["xrep"]:
                    nxt = states[r + 1]
                    fillers += setup_dma_pieces(nxt) + b0_q_pieces(nxt)
                    tails += b0_kv_pieces(nxt)
                attention(s, j, fillers, tails)
            if r + 1 < reps and not OPTS["xrep"]:
                for piece in setup_dma_pieces(states[r + 1]) + \
                        b0_q_pieces(states[r + 1]) + b0_kv_pieces(states[r + 1]):
                    piece()
            for piece in proj_pieces(s, TJ - 1):
                piece()

    nc.compile()
    return nc


def _get_nc(reps=1):
    key = f"nc{reps}"
    if key not in _CACHE:
        _CACHE[key] = _build_nc(reps)
    return _CACHE[key]


def make_in_maps(x, w_qkv, w_proj):
    """Shard full inputs into the 8 per-core input maps."""
    x = np.asarray(x, dtype=np.float32)
    w_qkv = np.asarray(w_qkv, dtype=np.float32)
    w_proj = np.asarray(w_proj, dtype=np.float32)
    row = np.arange(128)[:, None]
    col = np.arange(128)[None, :]
    bmask = (row <= col).astype(np.float16)
    amask = np.where(row <= col, np.float32(0.0),
                     np.float32(-1.0e30)).astype(np.float32)
    # per-pair head swap for w_proj rows: Y rows are [chB; chA]
    perm = np.concatenate([np.arange(p * 128 + 64, p * 128 + 128).tolist()
                           + np.arange(p * 128, p * 128 + 64).tolist()
                           for p in range(NPAIR)]).astype(np.int64)
    in_maps = []
    for c in range(8):
        b, hg = c // 2, c % 2
        sl = slice(hg * CO_Q, (hg + 1) * CO_Q)
        wpT = np.ascontiguousarray(w_proj[:, sl].T)[perm]
        in_maps.append({
            "xT": np.ascontiguousarray(x[b].T),
            "wqT": np.ascontiguousarray(w_qkv[0 * C:1 * C][sl].T),
            "wkT": np.ascontiguousarray(w_qkv[1 * C:2 * C][sl].T),
            "wvT": np.ascontiguousarray(w_qkv[2 * C:3 * C][sl].T),
            "wpT": np.ascontiguousarray(wpT),
            "bmask": bmask,
            "amask": amask,
        })
    return in_maps


def gather(results):
    """Sum the two head-group partials per batch, stack batches."""
    out = np.empty((B, T, C), dtype=np.float32)
    for b in range(B):
        out[b] = results[2 * b]["out"] + results[2 * b + 1]["out"]
    return out


def kernel(x, w_qkv, w_proj):
    nc = _get_nc()
    in_maps = make_in_maps(x, w_qkv, w_proj)
    res = run_bass_kernel_spmd(nc, in_maps, core_ids=list(range(8)))
    return gather(res.results)


# revision 12
# speedup vs baseline: 1.3995x; 1.3995x over previous
"""Causal self-attention (B=4, T=2048, C=1024, 16 heads) on 8 Trainium2 cores.

Sharding: batch x head-group.  Core c handles batch b = c//2 and head group
hg = c%2 (8 heads = 4 head-pairs).  Each core computes q/k/v projections for
its heads, causal attention, and a partial output projection; the host sums
the two partials per batch at gather.

v2 design (vs v1 at ~355-440us):
  * qkv/V projections run in float32r straight from DMA'd f32 inputs -- no
    staging or rounding copies for x and w_qkv (f32r matmuls are 1 cyc/row
    at N=512).  Attention stays fp16 (diag tiles have N=128 where f32r is
    4 cyc/row).
  * V is produced directly in [t, ch] layout by swapping matmul operands
    (lhsT = x^T tile), eliminating the PE transposes and vtmp copies of v1.
  * Per-si softmax: one 2-bank PSUM tile holds both heads' score strips;
    ONE activation instruction (2-region strided AP) does exp for both
    heads, halving ACT instruction overhead.
  * Causal mask: multiplicative fp16 0/1 mask applied post-exp on the SBUF
    probability tile (safe: scaled scores stay well under fp16 max), much
    cheaper than the f32 PSUM additive mask and off the score->exp chain.
  * Softmax denominators ride along as ones-columns in the V tiles
    ([onesA | VA | VB | onesB]); normalization uses GpSimd
    partition_broadcast (no DRAM round trip as in v1), unblocking the
    single-buffered y PSUM banks ~3us earlier per pair.
  * Software pipelining: the PE executes in emission order, so B(j+1)
    qkv groups and proj(j-1) groups are emitted interleaved between
    attention(j) si-groups to fill the PE bubbles left by the exp
    dependency (ACT is the per-si rate limiter during attention).
  * PSUM: qkv/V/proj share a 2-bank rotation, scores 2x2 banks, y 2 banks.

v4: cross-rep software pipelining.  attention(3) has 64 exp-bound si-groups
but the only legal same-rep filler is proj(<=2) (all qkv must precede it),
leaving the PE ~20us idle there.  With multiple reps in the NEFF (the
measured steady state, like consecutive transformer layers), rep r+1's
x/weight DMAs and B(0) q-projection groups are emitted into attention(3,r)'s
bubbles; k/V(0) groups follow during pair 3 once their WAR hazards clear
(earlier pairs still read kT(0)/vAB(0..3) at their si 0..3).  All reps share
one pool scope so persistent tiles chain rep-to-rep by WAR subtile deps.
The ones-column/mask init is also hoisted out of the rep loop.
"""
import numpy as np

import concourse.bass as bass
import concourse.tile as tile
from concourse import mybir, bacc
from concourse.bass_utils import run_bass_kernel_spmd

f32 = mybir.dt.float32
f32r = mybir.dt.float32r
f16 = mybir.dt.float16
Exp = mybir.ActivationFunctionType.Exp

B, T, C = 4, 2048, 1024
N_HEAD = 16
D = C // N_HEAD                 # 64
HPC = N_HEAD // 2               # heads per core = 8
NPAIR = HPC // 2                # head pairs per core = 4
CO_Q = C // 2                   # q channels per core = 512
CT = C // 128                   # contraction tiles for qkv = 8
TJ = T // 512                   # t super-tiles = 4
NS = T // 128                   # s tiles = 16
SCALE = float(D) ** -0.5        # 0.125

_CACHE = {}

# A/B knobs: mask: "mult" (fp16 post-exp multiplicative) | "add" (f32
# pre-exp additive); xrep: cross-rep pipelining of B(0) into attention(3).
OPTS = {"mask": "mult", "xrep": True}


def _reg2(t, col0, width, span=512):
    """2-region AP: columns [col0:col0+width] and [span+col0:span+col0+width]
    of a [128, 2*span] tile."""
    base = t[:, col0:col0 + 1]
    return bass.AP(tensor=base.tensor, offset=base.offset,
                   ap=[t.ap[0], [span, 2], [1, width]])


class _RepState:
    def __init__(self, rep):
        self.rep = rep
        self.xr = {}
        self.y = {}
        self.o = {}


def _build_nc(reps=1):
    import contextlib

    nc = bacc.Bacc("TRN2", target_bir_lowering=False, debug=False)
    xT_d = nc.dram_tensor("xT", [C, T], f32r, kind="ExternalInput").ap()
    wqT_d = nc.dram_tensor("wqT", [C, CO_Q], f32r, kind="ExternalInput").ap()
    wkT_d = nc.dram_tensor("wkT", [C, CO_Q], f32r, kind="ExternalInput").ap()
    wvT_d = nc.dram_tensor("wvT", [C, CO_Q], f32r, kind="ExternalInput").ap()
    wpT_d = nc.dram_tensor("wpT", [CO_Q, C], f32, kind="ExternalInput").ap()
    bmask_d = nc.dram_tensor("bmask", [128, 128], f16, kind="ExternalInput").ap()
    amask_d = nc.dram_tensor("amask", [128, 128], f32, kind="ExternalInput").ap()
    out_d = nc.dram_tensor("out", [T, C], f32, kind="ExternalOutput").ap()

    def dma(out, in_):
        nc.sync.dma_start(out=out, in_=in_)

    with tile.TileContext(nc) as tc, contextlib.ExitStack() as ctx:
        ep = ctx.enter_context
        persist = ep(tc.tile_pool(name="persist", bufs=1))
        xin = ep(tc.tile_pool(name="xin", bufs=2))
        wstg = ep(tc.tile_pool(name="wstg", bufs=2))
        pw = ep(tc.tile_pool(name="pw", bufs=4))
        ypool = ep(tc.tile_pool(name="ypool", bufs=2))
        rp = ep(tc.tile_pool(name="rp", bufs=2))
        ob = ep(tc.tile_pool(name="ob", bufs=2))
        qvp = ep(tc.tile_pool(name="qvp", bufs=2, space="PSUM"))
        sps = ep(tc.tile_pool(name="sps", bufs=2, space="PSUM"))
        ypsp = ep(tc.tile_pool(name="ypsp", bufs=1, space="PSUM"))

        # ---- persistent tensors, shared by all reps (WAR-chained) ----
        qT = persist.tile([128, NPAIR, TJ, 512], f16)
        kT = persist.tile([128, NPAIR, TJ, 512], f16)
        # vAB[:, p, si, :] = [onesA(64) | VA(64) | VB(64) | onesB(64)]
        vAB = persist.tile([128, NPAIR, NS, 256], f16)
        bmask = persist.tile([128, 128], f16)
        amask = persist.tile([128, 128], f32)
        wq_sb = persist.tile([128, CT, CO_Q], f32r)
        wk_sb = persist.tile([128, CT, CO_Q], f32r)
        wv_sb = persist.tile([128, CT, CO_Q], f32r)
        wpT_r = persist.tile([128, NPAIR, C], f16)
        onecol = persist.tile([128, 64], f16)

        # ---- one-time init (masks, ones-columns) ----
        dma(out=bmask[:], in_=bmask_d[:, :])
        if OPTS["mask"] == "add":
            dma(out=amask[:], in_=amask_d[:, :])
        nc.vector.memset(onecol[:], 0.0)
        nc.vector.memset(onecol[:, 0:1], 1.0)
        tmpl = bass.AP(tensor=onecol.tensor, offset=onecol.offset,
                       ap=[onecol.ap[0], [0, NPAIR], [0, NS], onecol.ap[1]])
        nc.vector.tensor_copy(vAB[:, :, :, 0:64], tmpl)
        nc.vector.tensor_copy(vAB[:, :, :, 192:256], tmpl)

        def emit_x_dma(s, j):
            xr = xin.tile([128, CT, 512], f32r, tag="xr", name=f"xr{s.rep}{j}")
            src = bass.AP(tensor=xT_d.tensor, offset=j * 512,
                          ap=[[T, 128], [128 * T, CT], [1, 512]])
            dma(out=xr[:], in_=src)
            s.xr[j] = xr

        def setup_dma_pieces(s):
            def wdmas():
                for wsb, wd in ((wq_sb, wqT_d), (wk_sb, wkT_d), (wv_sb, wvT_d)):
                    src = bass.AP(tensor=wd.tensor, offset=0,
                                  ap=[[CO_Q, 128], [128 * CO_Q, CT], [1, CO_Q]])
                    dma(out=wsb[:], in_=src)
            return [lambda: emit_x_dma(s, 0), wdmas]

        def wp_stage_pieces(s):
            pieces = []
            for p in range(NPAIR):
                def f(p=p):
                    ws = wstg.tile([128, C], f32, tag="ws", name=f"ws{s.rep}{p}")
                    dma(out=ws[:], in_=wpT_d[p * 128:(p + 1) * 128, :])
                    nc.vector.tensor_copy(wpT_r[:, p, :], ws[:])
                pieces.append(f)
            return pieces

        def b_group_pieces(s, j, kind, p_or_sj):
            """Closures for one qkv/V group: 4 matmul chunks + evict."""
            pieces = []
            if kind in ("q", "k"):
                i, dst, p = p_or_sj
                holder = {}

                def mk(ct0):
                    def f():
                        if ct0 == 0:
                            holder["ps"] = qvp.tile([128, 512], f32, tag="qv",
                                                    name=f"qk{s.rep}{j}{p}")
                        ps = holder["ps"]
                        wsb = (wq_sb, wk_sb)[i]
                        for ct in (ct0, ct0 + 1):
                            nc.tensor.matmul(
                                ps[:], wsb[:, ct, p * 128:(p + 1) * 128],
                                s.xr[j][:, ct, :],
                                start=(ct == 0), stop=(ct == CT - 1))
                    return f
                for ct0 in range(0, CT, 2):
                    pieces.append(mk(ct0))

                def ev():
                    nc.vector.tensor_copy(dst[:, p, j, :], holder["ps"][:])
                pieces.append(ev)
            else:  # V group: out [t-block, 512 vch]
                sj = p_or_sj
                holder = {}

                def mkv(ct0):
                    def f():
                        if ct0 == 0:
                            holder["ps"] = qvp.tile([128, 512], f32, tag="qv",
                                                    name=f"v{s.rep}{j}{sj}")
                        ps = holder["ps"]
                        for ct in (ct0, ct0 + 1):
                            nc.tensor.matmul(
                                ps[:], s.xr[j][:, ct, sj * 128:(sj + 1) * 128],
                                wv_sb[:, ct, :],
                                start=(ct == 0), stop=(ct == CT - 1))
                    return f
                for ct0 in range(0, CT, 2):
                    pieces.append(mkv(ct0))

                def evv():
                    si = j * 4 + sj
                    ps = holder["ps"]
                    for p in range(NPAIR):
                        nc.vector.tensor_copy(
                            vAB[:, p, si, 64:192], ps[:, p * 128:(p + 1) * 128])
                pieces.append(evv)
            return pieces

        def phase_b_pieces(s, j):
            pieces = [lambda: emit_x_dma(s, j)]
            for p in range(NPAIR):
                pieces += b_group_pieces(s, j, "q", (0, qT, p))
                pieces += b_group_pieces(s, j, "k", (1, kT, p))
            for sj in range(4):
                pieces += b_group_pieces(s, j, "v", sj)
            return pieces

        def b0_q_pieces(s):
            pieces = []
            for p in range(NPAIR):
                pieces += b_group_pieces(s, 0, "q", (0, qT, p))
            return pieces

        def b0_kv_pieces(s):
            pieces = []
            for p in range(NPAIR):
                pieces += b_group_pieces(s, 0, "k", (1, kT, p))
            for sj in range(4):
                pieces += b_group_pieces(s, 0, "v", sj)
            return pieces

        def proj_pieces(s, j):
            pieces = []
            for tj in range(4):
                for nh in range(2):
                    holder = {}

                    def mkp(p0, tj=tj, nh=nh, holder=holder):
                        def f():
                            if p0 == 0:
                                holder["ps"] = qvp.tile(
                                    [128, 512], f32, tag="qv",
                                    name=f"pr{s.rep}{j}{tj}{nh}")
                                if nh == 0:
                                    s.o[(j, tj)] = ob.tile(
                                        [128, C], f32, tag="o",
                                        name=f"o{s.rep}{j}{tj}")
                            ps = holder["ps"]
                            Y = s.y[j]
                            for p in (p0, p0 + 1):
                                nc.tensor.matmul(
                                    ps[:], Y[:, p, tj * 128:(tj + 1) * 128],
                                    wpT_r[:, p, nh * 512:(nh + 1) * 512],
                                    start=(p == 0), stop=(p == NPAIR - 1))
                        return f
                    pieces.append(mkp(0))
                    pieces.append(mkp(2))

                    def evp(tj=tj, nh=nh, holder=holder):
                        o_sb = s.o[(j, tj)]
                        nc.vector.tensor_copy(
                            o_sb[:, nh * 512:(nh + 1) * 512], holder["ps"][:])
                        if nh == 1:
                            row = j * 512 + tj * 128
                            dma(out=out_d[row:row + 128, :], in_=o_sb[:])
                    pieces.append(evp)
            return pieces

        def attention(s, j, fillers, tails):
            nsj = 4 * (j + 1)
            nslot = NPAIR * (nsj + 1)
            # tail fillers run only in pair 3 from si>=4 (their WAR hazards
            # against this rep's early-si reads clear there)
            ntail_slots = nsj - 4 + 1
            fi = 0
            ti = 0

            def pop(lst, idx, slots_left):
                want = len(lst) - idx
                if want <= 0:
                    return idx
                n = -(-want // max(slots_left, 1)) if slots_left > 0 else want
                for _ in range(n):
                    if idx < len(lst):
                        lst[idx]()
                        idx += 1
                return idx
            slot = nslot
            tslot = ntail_slots
            Y = ypool.tile([128, NPAIR, 512], f16, tag="Y", name=f"Y{s.rep}{j}")
            s.y[j] = Y
            for p in range(NPAIR):
                ypsA = ypsp.tile([128, 512], f32, tag="ypsA")
                ypsB = ypsp.tile([128, 512], f32, tag="ypsB")
                for si in range(nsj):
                    rel = si * 128 - j * 512
                    lo = max(rel, 0)
                    w = 512 - lo
                    stAB = sps.tile([128, 1024], f32, tag="st")
                    ko, ks = si // 4, (si % 4) * 128
                    nc.tensor.matmul(
                        stAB[:, lo:512], kT[0:64, p, ko, ks:ks + 128],
                        qT[0:64, p, j, lo:512], start=True, stop=True)
                    nc.tensor.matmul(
                        stAB[:, 512 + lo:1024], kT[64:128, p, ko, ks:ks + 128],
                        qT[64:128, p, j, lo:512], start=True, stop=True)
                    if rel >= 0 and OPTS["mask"] == "add":
                        mreg = _reg2(stAB, lo, 128)
                        msrc = bass.AP(tensor=amask.tensor, offset=amask.offset,
                                       ap=[amask.ap[0], [0, 2], [1, 128]])
                        nc.vector.tensor_add(mreg, mreg, msrc)
                    pAB = pw.tile([128, 1024], f16, tag="p")
                    nc.scalar.activation(_reg2(pAB, lo, w), _reg2(stAB, lo, w),
                                         Exp, scale=SCALE)
                    if rel >= 0 and OPTS["mask"] == "mult":
                        mreg = _reg2(pAB, lo, 128)
                        msrc = bass.AP(tensor=bmask.tensor, offset=bmask.offset,
                                       ap=[bmask.ap[0], [0, 2], [1, 128]])
                        nc.vector.tensor_mul(mreg, mreg, msrc)
                    st_f = (si == 0)
                    sp_f = (si == nsj - 1)
                    nc.tensor.matmul(ypsA[:, lo:512], vAB[:, p, si, 0:128],
                                     pAB[:, lo:512], start=st_f, stop=sp_f)
                    nc.tensor.matmul(ypsB[:, lo:512], vAB[:, p, si, 128:256],
                                     pAB[:, 512 + lo:1024], start=st_f, stop=sp_f)
                    slot -= 1
                    fi = pop(fillers, fi, slot)
                    if p == NPAIR - 1 and si >= 4:
                        tslot -= 1
                        ti = pop(tails, ti, tslot)
                # normalize: denomA at ypsA row 0, denomB at ypsB row 64
                rtA = rp.tile([1, 512], f32, tag="rtA")
                rtB = rp.tile([1, 512], f32, tag="rtB")
                nc.vector.reciprocal(rtA[0:1, :], ypsA[0:1, :])
                nc.vector.reciprocal(rtB[0:1, :], ypsB[64:65, :])
                rbA = rp.tile([128, 512], f32, tag="rbA")
                rbB = rp.tile([128, 512], f32, tag="rbB")
                nc.gpsimd.partition_broadcast(rbA[:], rtA[0:1, :])
                nc.gpsimd.partition_broadcast(rbB[:], rtB[0:1, :])
                nc.vector.tensor_mul(Y[64:128, p, :], ypsA[64:128, :],
                                     rbA[64:128, :])
                nc.vector.tensor_mul(Y[0:64, p, :], ypsB[0:64, :], rbB[0:64, :])
                slot -= 1
                fi = pop(fillers, fi, slot)
            while fi < len(fillers):
                fillers[fi]()
                fi += 1
            while ti < len(tails):
                tails[ti]()
                ti += 1

        # ---- schedule ----
        states = [_RepState(r) for r in range(reps)]
        for r in range(reps):
            s = states[r]
            if r == 0:
                for piece in setup_dma_pieces(s) + wp_stage_pieces(s):
                    piece()
                for piece in b0_q_pieces(s) + b0_kv_pieces(s):
                    piece()
            else:
                for piece in wp_stage_pieces(s):
                    piece()
            for j in range(TJ):
                fillers, tails = [], []
                if j + 1 < TJ:
                    fillers += phase_b_pieces(s, j + 1)
                if j - 1 >= 0:
                    fillers += proj_pieces(s, j - 1)
                if j == TJ - 1 and r + 1 < reps and OPTS["xrep"]:
                    nxt = states[r + 1]
                    fillers += setup_dma_pieces(nxt) + b0_q_pieces(nxt)
                    tails += b0_kv_pieces(nxt)
                attention(s, j, fillers, tails)
            if r + 1 < reps and not OPTS["xrep"]:
                for piece in setup_dma_pieces(states[r + 1]) + \
                        b0_q_pieces(states[r + 1]) + b0_kv_pieces(states[r + 1]):
                    piece()
            for piece in proj_pieces(s, TJ - 1):
                piece()

    nc.compile()
    return nc


def _get_nc(reps=1):
    key = f"nc{reps}"
    if key not in _CACHE:
        _CACHE[key] = _build_nc(reps)
    return _CACHE[key]


def make_in_maps(x, w_qkv, w_proj):
    """Shard full inputs into the 8 per-core input maps."""
    x = np.asarray(x, dtype=np.float32)
    w_qkv = np.asarray(w_qkv, dtype=np.float32)
    w_proj = np.asarray(w_proj, dtype=np.float32)
    row = np.arange(128)[:, None]
    col = np.arange(128)[None, :]
    bmask = (row <= col).astype(np.float16)
    amask = np.where(row <= col, np.float32(0.0),
                     np.float32(-1.0e30)).astype(np.float32)
    # per-pair head swap for w_proj rows: Y rows are [chB; chA]
    perm = np.concatenate([np.arange(p * 128 + 64, p * 128 + 128).tolist()
                           + np.arange(p * 128, p * 128 + 64).tolist()
                           for p in range(NPAIR)]).astype(np.int64)
    in_maps = []
    for c in range(8):
        b, hg = c // 2, c % 2
        sl = slice(hg * CO_Q, (hg + 1) * CO_Q)
        wpT = np.ascontiguousarray(w_proj[:, sl].T)[perm]
        in_maps.append({
            "xT": np.ascontiguousarray(x[b].T),
            "wqT": np.ascontiguousarray(w_qkv[0 * C:1 * C][sl].T),
            "wkT": np.ascontiguousarray(w_qkv[1 * C:2 * C][sl].T),
            "wvT": np.ascontiguousarray(w_qkv[2 * C:3 * C][sl].T),
            "wpT": np.ascontiguousarray(wpT),
            "bmask": bmask,
            "amask": amask,
        })
    return in_maps


def gather(results):
    """Sum the two head-group partials per batch, stack batches."""
    out = np.empty((B, T, C), dtype=np.float32)
    for b in range(B):
        out[b] = results[2 * b]["out"] + results[2 * b + 1]["out"]
    return out


def kernel(x, w_qkv, w_proj):
    nc = _get_nc()
    in_maps = make_in_maps(x, w_qkv, w_proj)
    res = run_bass_kernel_spmd(nc, in_maps, core_ids=list(range(8)))
    return gather(res.results)
